# revision 3
# baseline (speedup 1.0000x reference)
"""GCN (7-layer, PyG GCNConv-style) on 8 Trainium2 NeuronCores.

Strategy (graph-partition data parallel, per sharding hint):
- Nodes are destination-sharded contiguously: core k owns nodes
  [k*12500, (k+1)*12500). Each core aggregates messages for its own nodes.
- Per layer: each core computes h~ = dinv * (H @ W) for its local nodes,
  AllGathers the full node-feature table to DRAM, then for each 128-edge
  chunk (edges sorted by destination block) gathers source rows with a
  native indirect DMA, scales by edge weight, and accumulates into PSUM via
  a selection-matrix matmul on the TensorEngine (S[e, d] = 1 if edge e's
  destination-in-block == d). Self-loops fold in algebraically:
  out = dinv*(agg + h~) + b, since dinv*h~ = dinv^2*h.
- Degrees (1 + sum of incoming edge weights) are computed on device by row
  reduction over a host-permuted, zero-padded copy of edge_weight;
  dinv = rsqrt(deg) on the scalar engine.

Host-side work is index/layout preparation only (sharding, edge sorting,
padding); all floating-point math runs on device.
"""
import sys

sys.path.insert(0, "/opt/trn_rl_repo")

from contextlib import ExitStack

import numpy as np

NC = 8
N_NODES = 100000
NLOC = N_NODES // NC            # 12500
NBLK = (NLOC + 127) // 128      # 98
NLOCP = NBLK * 128              # 12544 (padded local nodes)
NTAB = NC * NLOCP               # padded global table rows
DIMS = [(128, 50), (50, 50), (50, 30), (30, 30), (30, 10), (10, 10), (10, 1)]
NLAYER = len(DIMS)


def _host_prep(x, edge_index, edge_weight):
    """Shard + sort edges, build per-core device input arrays."""
    row = np.asarray(edge_index[0], dtype=np.int64)
    col = np.asarray(edge_index[1], dtype=np.int64)
    w = np.asarray(edge_weight, dtype=np.float32)

    core_of = col // NLOC
    per_core = []
    blk_cnt_max = np.zeros(NBLK, np.int64)
    max_deg = 1
    for k in range(NC):
        m = core_of == k
        r_k = row[m]
        c_k = col[m] - k * NLOC
        w_k = w[m]
        blk = c_k // 128
        pos = c_k % 128
        order = np.argsort(blk, kind="stable")
        r_k, c_k, w_k, blk, pos = (a[order] for a in (r_k, c_k, w_k, blk, pos))
        cnt = np.bincount(blk, minlength=NBLK)
        blk_cnt_max = np.maximum(blk_cnt_max, cnt)
        degc = np.bincount(c_k, weights=w_k, minlength=NLOCP)
        # in-degree count for wpad sizing
        cdeg = np.bincount(c_k, minlength=NLOCP)
        max_deg = max(max_deg, int(cdeg.max()))
        per_core.append((r_k, c_k, w_k, blk, pos, cnt, cdeg))

    M_b = np.maximum(1, np.ceil(blk_cnt_max / 128).astype(np.int64))
    cum = np.zeros(NBLK + 1, np.int64)
    cum[1:] = np.cumsum(M_b)
    NCH = int(cum[-1])
    D = max_deg

    in_maps = []
    for k in range(NC):
        r_k, c_k, w_k, blk, pos, cnt, cdeg = per_core[k]
        n_e = len(r_k)
        # slot within the destination block's edge run
        first = np.zeros(NBLK + 1, dtype=np.int64)
        first[1:] = np.cumsum(cnt)
        rank = np.arange(n_e, dtype=np.int64) - first[blk]
        chunk = cum[blk] + rank // 128
        part = rank % 128

        g_idx = np.zeros((128, NCH), np.int32)
        dst_pos = np.zeros((128, NCH), np.float32)
        w_e = np.zeros((128, NCH), np.float32)
        # padded global table id of the source node (partition-major layout:
        # core*NLOCP + (local%128)*NBLK + local//128, matching the p-major bounce)
        loc = r_k % NLOC
        src_pad = (r_k // NLOC) * NLOCP + (loc % 128) * NBLK + loc // 128
        g_idx[part, chunk] = src_pad.astype(np.int32)
        dst_pos[part, chunk] = pos.astype(np.float32)
        w_e[part, chunk] = w_k

        # padded per-node incoming weights for degree computation
        order2 = np.argsort(c_k, kind="stable")
        c_s = c_k[order2]
        w_s = w_k[order2]
        nfirst = np.zeros(NLOCP + 1, np.int64)
        nfirst[1:] = np.cumsum(np.bincount(c_s, minlength=NLOCP))
        nrank = np.arange(len(c_s), dtype=np.int64) - nfirst[c_s]
        wpad = np.zeros((NLOCP, D), np.float32)
        wpad[c_s, nrank] = w_s
        # device layout [128, NBLK, D]: node c*128+p -> [p, c, :]
        wpad_dev = wpad.reshape(NBLK, 128, D).transpose(1, 0, 2).copy()

        xk = np.zeros((NLOCP, x.shape[1]), np.float32)
        xk[:NLOC] = x[k * NLOC : (k + 1) * NLOC]

        in_maps.append(
            {
                "x_p": xk,
                "g_idx": g_idx,
                "dst_pos": dst_pos,
                "w_e": w_e,
                "wpad": wpad_dev,
            }
        )
    return in_maps, M_b, cum, D


def _build_program(M_b, cum, D, weights_shapes):
    from concourse import bass, bacc, mybir, tile

    NCH = int(cum[-1])
    nc = bacc.Bacc("TRN2", target_bir_lowering=False, debug=False, num_devices=NC)

    x_p = nc.declare_dram_parameter("x_p", [NLOCP, 128], mybir.dt.float32, isOutput=False)
    g_idx = nc.declare_dram_parameter("g_idx", [128, NCH], mybir.dt.int32, isOutput=False)
    dst_pos = nc.declare_dram_parameter("dst_pos", [128, NCH], mybir.dt.float32, isOutput=False)
    w_e = nc.declare_dram_parameter("w_e", [128, NCH], mybir.dt.float32, isOutput=False)
    wpad = nc.declare_dram_parameter("wpad", [128, NBLK, D], mybir.dt.float32, isOutput=False)
    iota = nc.declare_dram_parameter("iota", [128, 128], mybir.dt.float32, isOutput=False)
    ident = nc.declare_dram_parameter("ident", [128, 128], mybir.dt.float32, isOutput=False)
    Ws, Bs = [], []
    for i, (fi, fo) in enumerate(DIMS):
        Ws.append(nc.declare_dram_parameter(f"W{i+1}", [fi, fo], mybir.dt.float32, isOutput=False))
        Bs.append(nc.declare_dram_parameter(f"b{i+1}", [128, fo], mybir.dt.float32, isOutput=False))
    out_ext = nc.declare_dram_parameter("out", [128, NBLK], mybir.dt.float32, isOutput=True)

    bounces = [nc.dram_tensor(f"bounce{i}", [128, NBLK * fo], mybir.dt.float32) for i, (fi, fo) in enumerate(DIMS)]
    tables = [
        nc.dram_tensor(f"table{i}", [NC * 128, NBLK * fo], mybir.dt.float32, addr_space="Shared")
        for i, (fi, fo) in enumerate(DIMS)
    ]

    with tile.TileContext(nc) as tc, ExitStack() as ctx:
        const = ctx.enter_context(tc.tile_pool(name="const", bufs=1))
        work = ctx.enter_context(tc.tile_pool(name="work", bufs=2))
        wpool = ctx.enter_context(tc.tile_pool(name="wpool", bufs=1))
        hpool = ctx.enter_context(tc.tile_pool(name="hpool", bufs=2))
        msgp = ctx.enter_context(tc.tile_pool(name="msgp", bufs=24))
        selp = ctx.enter_context(tc.tile_pool(name="selp", bufs=12))
        psT = ctx.enter_context(tc.tile_pool(name="psT", bufs=2, space="PSUM"))
        psH = ctx.enter_context(tc.tile_pool(name="psH", bufs=2, space="PSUM"))
        psA = ctx.enter_context(tc.tile_pool(name="psA", bufs=4, space="PSUM"))

        iota_t = const.tile([128, 128], mybir.dt.float32)
        nc.sync.dma_start(out=iota_t[:], in_=iota[:])
        ident_t = const.tile([128, 128], mybir.dt.float32)
        nc.sync.dma_start(out=ident_t[:], in_=ident[:])
        idx_t = const.tile([128, NCH], mybir.dt.int32)
        nc.sync.dma_start(out=idx_t[:], in_=g_idx[:])
        pos_t = const.tile([128, NCH], mybir.dt.float32)
        nc.sync.dma_start(out=pos_t[:], in_=dst_pos[:])
        wgt_t = const.tile([128, NCH], mybir.dt.float32)
        nc.sync.dma_start(out=wgt_t[:], in_=w_e[:])
        W_ts, B_ts = [], []
        for i, (fi, fo) in enumerate(DIMS):
            W_t = const.tile([fi, fo], mybir.dt.float32, tag=f"W{i}")
            nc.sync.dma_start(out=W_t[:], in_=Ws[i][:])
            B_t = const.tile([128, fo], mybir.dt.float32, tag=f"B{i}")
            nc.sync.dma_start(out=B_t[:], in_=Bs[i][:])
            W_ts.append(W_t)
            B_ts.append(B_t)

        # ---- degree -> dinv ----
        wpad_t = wpool.tile([128, NBLK, D], mybir.dt.float32, tag="wpad")
        nc.sync.dma_start(out=wpad_t[:], in_=wpad[:])
        deg_t = const.tile([128, NBLK], mybir.dt.float32)
        for c in range(NBLK):
            nc.vector.tensor_reduce(
                deg_t[:, c : c + 1],
                wpad_t[:, c, :],
                mybir.AxisListType.X,
                mybir.AluOpType.add,
            )
        sqrt_t = const.tile([128, NBLK], mybir.dt.float32)
        # dinv = 1 / sqrt(deg + 1)  (+1 = self-loop weight)
        nc.scalar.activation(
            out=sqrt_t[:], in_=deg_t[:], func=mybir.ActivationFunctionType.Sqrt, bias=1.0, scale=1.0
        )
        dinv_t = const.tile([128, NBLK], mybir.dt.float32)
        nc.vector.reciprocal(out=dinv_t[:], in_=sqrt_t[:])

        h_cur = None  # SBUF tile [128, NBLK, F_in] for layers >= 2
        for li, (fi, fo) in enumerate(DIMS):
            last = li == NLAYER - 1
            htil = hpool.tile([128, NBLK, fo], mybir.dt.float32, tag="htil")
            # ---- stage A: h~ = dinv * (H @ W) ----
            for c in range(NBLK):
                if li == 0:
                    h_chunk = work.tile([128, fi], mybir.dt.float32, tag="xchunk")
                    nc.sync.dma_start(
                        out=h_chunk[:],
                        in_=x_p[:].rearrange("(c p) f -> p c f", p=128)[:, c, :],
                    )
                    src_ap = h_chunk[:]
                else:
                    src_ap = h_cur[:, c, :]
                pT = psT.tile([fi, 128], mybir.dt.float32, space="PSUM", tag="pT")
                nc.tensor.transpose(out=pT[:], in_=src_ap, identity=ident_t[:])
                hT = work.tile([fi, 128], mybir.dt.float32, tag="hT")
                nc.vector.tensor_copy(out=hT[:], in_=pT[:])
                pH = psH.tile([128, fo], mybir.dt.float32, space="PSUM", tag="pH")
                nc.tensor.matmul(out=pH[:], lhsT=hT[:], rhs=W_ts[li][:], start=True, stop=True)
                nc.scalar.activation(
                    out=htil[:, c, :],
                    in_=pH[:],
                    func=mybir.ActivationFunctionType.Copy,
                    scale=dinv_t[:, c : c + 1],
                )
            # ---- stage B/C: allgather h~ table ----
            nc.sync.dma_start(out=bounces[li][:], in_=htil[:])
            nc.gpsimd.collective_compute(
                "AllGather",
                mybir.AluOpType.bypass,
                ins=[bounces[li][:]],
                outs=[tables[li][:]],
                replica_groups=[list(range(NC))],
            )
            # ---- stage D: aggregate ----
            h_next = (
                None
                if last
                else hpool.tile([128, NBLK, fo], mybir.dt.float32, tag="h")
            )
            if last:
                out_sb = work.tile([128, NBLK], mybir.dt.float32, tag="outsb")
            for b in range(NBLK):
                acc = psA.tile([128, fo], mybir.dt.float32, space="PSUM", tag="acc")
                Mb = int(M_b[b])
                for j in range(Mb):
                    cc = int(cum[b]) + j
                    msg = msgp.tile([128, fo], mybir.dt.float32, tag="msg")
                    nc.gpsimd.indirect_dma_start(
                        out=msg[:],
                        out_offset=None,
                        in_=tables[li][:].rearrange("r (c f) -> (r c) f", f=fo),
                        in_offset=bass.IndirectOffsetOnAxis(ap=idx_t[:, cc : cc + 1], axis=0),
                    )
                    nc.vector.tensor_scalar_mul(msg[:], msg[:], wgt_t[:, cc : cc + 1])
                    S = selp.tile([128, 128], mybir.dt.float32, tag="S")
                    nc.vector.tensor_tensor(
                        out=S[:],
                        in0=iota_t[:],
                        in1=pos_t[:, cc : cc + 1].to_broadcast([128, 128]),
                        op=mybir.AluOpType.is_equal,
                    )
                    nc.tensor.matmul(out=acc[:], lhsT=S[:], rhs=msg[:], start=(j == 0), stop=(j == Mb - 1))
                # out_b = dinv*(acc + h~) + b ; relu unless last layer
                t1 = work.tile([128, fo], mybir.dt.float32, tag="t1")
                nc.vector.tensor_tensor(out=t1[:], in0=acc[:], in1=htil[:, b, :], op=mybir.AluOpType.add)
                dst = out_sb[:, b : b + 1] if last else h_next[:, b, :]
                t2 = work.tile([128, fo], mybir.dt.float32, tag="t2")
                nc.vector.scalar_tensor_tensor(
                    out=t2[:],
                    in0=t1[:],
                    scalar=dinv_t[:, b : b + 1],
                    in1=B_ts[li][:],
                    op0=mybir.AluOpType.mult,
                    op1=mybir.AluOpType.add,
                )
                if last:
                    nc.vector.tensor_copy(out=dst, in_=t2[:])
                else:
                    nc.scalar.activation(out=dst, in_=t2[:], func=mybir.ActivationFunctionType.Relu)
            h_cur = h_next
        nc.sync.dma_start(out=out_ext[:], in_=out_sb[:])

    nc.finalize()
    return nc


LAST_EXEC_NS = None
LAST_RESULT = None


def kernel(x, edge_index, edge_weight, W1, b1, W2, b2, W3, b3, W4, b4, W5, b5, W6, b6, W7, b7):
    global LAST_EXEC_NS, LAST_RESULT
    import os

    from concourse.bass_utils import run_bass_kernel_spmd

    x = np.asarray(x, dtype=np.float32)
    in_maps, M_b, cum, D = _host_prep(x, np.asarray(edge_index), np.asarray(edge_weight))

    Wmats = [np.asarray(W, dtype=np.float32) for W in (W1, W2, W3, W4, W5, W6, W7)]
    bvecs = [np.tile(np.asarray(b, dtype=np.float32).reshape(1, -1), (128, 1)) for b in (b1, b2, b3, b4, b5, b6, b7)]
    iota = np.tile(np.arange(128, dtype=np.float32).reshape(1, 128), (128, 1))
    ident = np.eye(128, dtype=np.float32)
    for m in in_maps:
        for i in range(NLAYER):
            m[f"W{i+1}"] = Wmats[i]
            m[f"b{i+1}"] = bvecs[i]
        m["iota"] = iota
        m["ident"] = ident

    nc = _build_program(M_b, cum, D, None)
    trace = bool(int(os.environ.get("GCN_TRACE", "0")))
    res = run_bass_kernel_spmd(
        nc,
        in_maps,
        list(range(NC)),
        trace=trace,
        tmpdir=os.environ.get("GCN_TRACE_DIR") or None,
    )
    LAST_EXEC_NS = res.exec_time_ns
    LAST_RESULT = res
    outs = []
    for k in range(NC):
        pm = res.results[k]["out"]  # [128, NBLK], node c*128+p at [p, c]
        outs.append(pm.T.reshape(-1, 1)[:NLOC])
    out = np.concatenate(outs, axis=0)
    return out



# revision 7
# speedup vs baseline: 1.0061x; 1.0061x over previous
"""GCN (7-layer, PyG GCNConv-style) on 8 Trainium2 NeuronCores.

Strategy (graph-partition data parallel, per sharding hint):
- Nodes are destination-sharded contiguously: core k owns nodes
  [k*12500, (k+1)*12500). Each core aggregates messages for its own nodes.
- Per layer: each core computes h~ = dinv * (H @ W) for its local nodes,
  AllGathers the full node-feature table to DRAM, then for each 128-edge
  chunk (edges sorted by destination block) gathers source rows with a
  native indirect DMA, scales by edge weight (casting to bf16), and
  accumulates into PSUM via a bf16 selection-matrix matmul on the
  TensorEngine (S[e, d] = 1 if edge e's destination-in-block == d).
  Self-loops fold in algebraically: out = dinv*(agg + h~) + b.
- The SWDGE descriptor generation for the per-chunk gathers is the
  throughput bound; all compute (PE matmuls in bf16 with PSUM chains
  interleaved across block pairs, DVE selection builds and scaling,
  per-block epilogue + next-layer projection) is pipelined under it.
- Degrees (1 + sum of incoming edge weights) are computed on device by row
  reduction over a host-permuted, zero-padded copy of edge_weight;
  dinv = rsqrt(deg) on the scalar engine.

Host-side work is index/layout preparation only (sharding, edge sorting,
padding, dtype casts); all floating-point math runs on device.
"""
import sys

sys.path.insert(0, "/opt/trn_rl_repo")

from contextlib import ExitStack

import ml_dtypes
import numpy as np

NC = 8
N_NODES = 100000
NLOC = N_NODES // NC            # 12500
NBLK = (NLOC + 127) // 128      # 98
NLOCP = NBLK * 128              # 12544 (padded local nodes)
NTAB = NC * NLOCP               # padded global table rows
DIMS = [(128, 50), (50, 50), (50, 30), (30, 30), (30, 10), (10, 10), (10, 1)]
NLAYER = len(DIMS)


def _host_prep(x, edge_index, edge_weight):
    """Shard + sort edges, build per-core device input arrays."""
    row = np.asarray(edge_index[0], dtype=np.int64)
    col = np.asarray(edge_index[1], dtype=np.int64)
    w = np.asarray(edge_weight, dtype=np.float32)

    core_of = col // NLOC
    per_core = []
    blk_cnt_max = np.zeros(NBLK, np.int64)
    max_deg = 1
    for k in range(NC):
        m = core_of == k
        r_k = row[m]
        c_k = col[m] - k * NLOC
        w_k = w[m]
        blk = c_k // 128
        pos = c_k % 128
        order = np.argsort(blk, kind="stable")
        r_k, c_k, w_k, blk, pos = (a[order] for a in (r_k, c_k, w_k, blk, pos))
        cnt = np.bincount(blk, minlength=NBLK)
        blk_cnt_max = np.maximum(blk_cnt_max, cnt)
        degc = np.bincount(c_k, weights=w_k, minlength=NLOCP)
        cdeg = np.bincount(c_k, minlength=NLOCP)
        max_deg = max(max_deg, int(cdeg.max()))
        per_core.append((r_k, c_k, w_k, blk, pos, cnt, cdeg))

    M_b = np.maximum(1, np.ceil(blk_cnt_max / 128).astype(np.int64))
    cum = np.zeros(NBLK + 1, np.int64)
    cum[1:] = np.cumsum(M_b)
    NCH = int(cum[-1])
    D = max_deg

    in_maps = []
    for k in range(NC):
        r_k, c_k, w_k, blk, pos, cnt, cdeg = per_core[k]
        n_e = len(r_k)
        first = np.zeros(NBLK + 1, dtype=np.int64)
        first[1:] = np.cumsum(cnt)
        rank = np.arange(n_e, dtype=np.int64) - first[blk]
        chunk = cum[blk] + rank // 128
        part = rank % 128

        g_idx = np.zeros((128, NCH), np.int32)
        dst_pos = np.zeros((128, NCH), np.float32)
        w_e = np.zeros((128, NCH), np.float32)
        # padded global table id of the source node (partition-major layout:
        # core*NLOCP + (local%128)*NBLK + local//128, matching the p-major bounce)
        loc = r_k % NLOC
        src_pad = (r_k // NLOC) * NLOCP + (loc % 128) * NBLK + loc // 128
        g_idx[part, chunk] = src_pad.astype(np.int32)
        dst_pos[part, chunk] = pos.astype(np.float32)
        w_e[part, chunk] = w_k

        # padded per-node incoming weights for degree computation
        order2 = np.argsort(c_k, kind="stable")
        c_s = c_k[order2]
        w_s = w_k[order2]
        nfirst = np.zeros(NLOCP + 1, np.int64)
        nfirst[1:] = np.cumsum(np.bincount(c_s, minlength=NLOCP))
        nrank = np.arange(len(c_s), dtype=np.int64) - nfirst[c_s]
        wpad = np.zeros((NLOCP, D), np.float32)
        wpad[c_s, nrank] = w_s
        wpad_dev = wpad.reshape(NBLK, 128, D).transpose(1, 0, 2).copy()

        # x transposed per block for direct use as stage-A stationary operand:
        # xT[f, b, n] = x[k*NLOC + b*128 + n, f]
        xk = np.zeros((NLOCP, x.shape[1]), np.float32)
        xk[:NLOC] = x[k * NLOC : (k + 1) * NLOC]
        xT = xk.reshape(NBLK, 128, x.shape[1]).transpose(2, 0, 1).copy()

        in_maps.append(
            {
                "xT": xT.astype(ml_dtypes.bfloat16),
                "g_idx": g_idx,
                "dst_pos": dst_pos.astype(ml_dtypes.bfloat16),
                "w_e": w_e,
                "wpad": wpad_dev,
            }
        )
    return in_maps, M_b, cum, D


def _build_program(M_b, cum, D):
    from concourse import bass, bacc, mybir, tile

    NCH = int(cum[-1])
    nc = bacc.Bacc("TRN2", target_bir_lowering=False, debug=False, num_devices=NC)

    xT_p = nc.declare_dram_parameter("xT", [128, NBLK, 128], mybir.dt.bfloat16, isOutput=False)
    g_idx = nc.declare_dram_parameter("g_idx", [128, NCH], mybir.dt.int32, isOutput=False)
    dst_pos = nc.declare_dram_parameter("dst_pos", [128, NCH], mybir.dt.bfloat16, isOutput=False)
    w_e = nc.declare_dram_parameter("w_e", [128, NCH], mybir.dt.float32, isOutput=False)
    wpad = nc.declare_dram_parameter("wpad", [128, NBLK, D], mybir.dt.float32, isOutput=False)
    iota = nc.declare_dram_parameter("iota", [128, 128], mybir.dt.bfloat16, isOutput=False)
    ident = nc.declare_dram_parameter("ident", [128, 128], mybir.dt.bfloat16, isOutput=False)
    Ws, Bs = [], []
    for i, (fi, fo) in enumerate(DIMS):
        Ws.append(nc.declare_dram_parameter(f"W{i+1}", [fi, fo], mybir.dt.bfloat16, isOutput=False))
        Bs.append(nc.declare_dram_parameter(f"b{i+1}", [128, fo], mybir.dt.float32, isOutput=False))
    out_ext = nc.declare_dram_parameter("out", [128, NBLK], mybir.dt.float32, isOutput=True)

    bounces = [nc.dram_tensor(f"bounce{i}", [128, NBLK * fo], mybir.dt.float32) for i, (fi, fo) in enumerate(DIMS)]
    tables = [
        nc.dram_tensor(f"table{i}", [NC * 128, NBLK * fo], mybir.dt.float32, addr_space="Shared")
        for i, (fi, fo) in enumerate(DIMS)
    ]

    with tile.TileContext(nc) as tc, ExitStack() as ctx:
        const = ctx.enter_context(tc.tile_pool(name="const", bufs=1))
        work = ctx.enter_context(tc.tile_pool(name="work", bufs=4))
        wpool = ctx.enter_context(tc.tile_pool(name="wpool", bufs=1))
        hpool = ctx.enter_context(tc.tile_pool(name="hpool", bufs=2))
        msgp = ctx.enter_context(tc.tile_pool(name="msgp", bufs=24))
        msgh = ctx.enter_context(tc.tile_pool(name="msgh", bufs=24))
        selp = ctx.enter_context(tc.tile_pool(name="selp", bufs=12))
        psT = ctx.enter_context(tc.tile_pool(name="psT", bufs=2, space="PSUM"))
        psH = ctx.enter_context(tc.tile_pool(name="psH", bufs=2, space="PSUM"))
        psA = ctx.enter_context(tc.tile_pool(name="psA", bufs=2, space="PSUM"))

        iota_t = const.tile([128, 128], mybir.dt.bfloat16)
        nc.sync.dma_start(out=iota_t[:], in_=iota[:])
        ident_t = const.tile([128, 128], mybir.dt.bfloat16)
        nc.sync.dma_start(out=ident_t[:], in_=ident[:])
        idx_t = const.tile([128, NCH], mybir.dt.int32)
        nc.sync.dma_start(out=idx_t[:], in_=g_idx[:])
        pos_t = const.tile([128, NCH], mybir.dt.bfloat16)
        nc.sync.dma_start(out=pos_t[:], in_=dst_pos[:])
        wgt_t = const.tile([128, NCH], mybir.dt.float32)
        nc.sync.dma_start(out=wgt_t[:], in_=w_e[:])
        xT_t = const.tile([128, NBLK, 128], mybir.dt.bfloat16)
        nc.sync.dma_start(out=xT_t[:], in_=xT_p[:])
        W_ts, B_ts = [], []
        for i, (fi, fo) in enumerate(DIMS):
            W_t = const.tile([fi, fo], mybir.dt.bfloat16, tag=f"W{i}")
            nc.sync.dma_start(out=W_t[:], in_=Ws[i][:])
            B_t = const.tile([128, fo], mybir.dt.float32, tag=f"B{i}")
            nc.sync.dma_start(out=B_t[:], in_=Bs[i][:])
            W_ts.append(W_t)
            B_ts.append(B_t)

        # ---- degree -> dinv ----
        wpad_t = wpool.tile([128, NBLK, D], mybir.dt.float32, tag="wpad")
        nc.sync.dma_start(out=wpad_t[:], in_=wpad[:])
        deg_t = const.tile([128, NBLK], mybir.dt.float32)
        for c in range(NBLK):
            nc.vector.tensor_reduce(
                deg_t[:, c : c + 1],
                wpad_t[:, c, :],
                mybir.AxisListType.X,
                mybir.AluOpType.add,
            )
        sqrt_t = const.tile([128, NBLK], mybir.dt.float32)
        # dinv = 1 / sqrt(deg + 1)  (+1 = self-loop weight)
        nc.scalar.activation(
            out=sqrt_t[:], in_=deg_t[:], func=mybir.ActivationFunctionType.Sqrt, bias=1.0, scale=1.0
        )
        dinv_t = const.tile([128, NBLK], mybir.dt.float32)
        nc.vector.reciprocal(out=dinv_t[:], in_=sqrt_t[:])

        def stage_a(li, b, lhsT_ap, htil):
            """htil_li[b] = dinv * (H @ W_li) from the transposed block operand."""
            fi, fo = DIMS[li]
            pH = psH.tile([128, fo], mybir.dt.float32, space="PSUM", tag="pH")
            nc.tensor.matmul(out=pH[:], lhsT=lhsT_ap, rhs=W_ts[li][:], start=True, stop=True)
            nc.scalar.activation(
                out=htil[:, b, :],
                in_=pH[:],
                func=mybir.ActivationFunctionType.Copy,
                scale=dinv_t[:, b : b + 1],
            )

        # ---- layer 1 stage A (from host-transposed x) ----
        htil_cur = hpool.tile([128, NBLK, DIMS[0][1]], mybir.dt.float32, tag="h0")
        for b in range(NBLK):
            stage_a(0, b, xT_t[:, b, :], htil_cur)

        out_sb = const.tile([128, NBLK], mybir.dt.float32)

        for li, (fi, fo) in enumerate(DIMS):
            last = li == NLAYER - 1
            # ---- publish table: bounce write + AllGather ----
            nc.sync.dma_start(out=bounces[li][:], in_=htil_cur[:])
            nc.gpsimd.collective_compute(
                "AllGather",
                mybir.AluOpType.bypass,
                ins=[bounces[li][:]],
                outs=[tables[li][:]],
                replica_groups=[list(range(NC))],
            )
            htil_next = (
                None
                if last
                else hpool.tile([128, NBLK, DIMS[li + 1][1]], mybir.dt.float32, tag=f"h{(li+1)%2}")
            )

            table_rows = tables[li][:].rearrange("r (c f) -> (r c) f", f=fo)

            def emit_chunk(b, j):
                cc = int(cum[b]) + j
                msg = msgp.tile([128, fo], mybir.dt.float32, tag="msg")
                nc.gpsimd.indirect_dma_start(
                    out=msg[:],
                    out_offset=None,
                    in_=table_rows,
                    in_offset=bass.IndirectOffsetOnAxis(ap=idx_t[:, cc : cc + 1], axis=0),
                )
                mh = msgh.tile([128, fo], mybir.dt.bfloat16, tag="mh")
                nc.vector.tensor_scalar_mul(mh[:], msg[:], wgt_t[:, cc : cc + 1])
                S = selp.tile([128, 128], mybir.dt.bfloat16, tag="S")
                nc.vector.tensor_tensor(
                    out=S[:],
                    in0=iota_t[:],
                    in1=pos_t[:, cc : cc + 1].to_broadcast([128, 128]),
                    op=mybir.AluOpType.is_equal,
                )
                return S, mh

            def finish_block(b, acc):
                # out_b = dinv*(acc + h~) + bias ; relu unless last layer
                t1 = work.tile([128, fo], mybir.dt.float32, tag="t1")
                nc.vector.tensor_tensor(out=t1[:], in0=acc[:], in1=htil_cur[:, b, :], op=mybir.AluOpType.add)
                t2 = work.tile([128, fo], mybir.dt.float32, tag="t2")
                nc.vector.scalar_tensor_tensor(
                    out=t2[:],
                    in0=t1[:],
                    scalar=dinv_t[:, b : b + 1],
                    in1=B_ts[li][:],
                    op0=mybir.AluOpType.mult,
                    op1=mybir.AluOpType.add,
                )
                if last:
                    nc.vector.tensor_copy(out=out_sb[:, b : b + 1], in_=t2[:])
                    return
                hn = work.tile([128, fo], mybir.dt.bfloat16, tag="hn")
                nc.scalar.activation(out=hn[:], in_=t2[:], func=mybir.ActivationFunctionType.Relu)
                # transpose for the next layer's stage-A stationary operand
                pT = psT.tile([fo, 128], mybir.dt.bfloat16, space="PSUM", tag="pT")
                nc.tensor.transpose(out=pT[:], in_=hn[:], identity=ident_t[:])
                hT = work.tile([fo, 128], mybir.dt.bfloat16, tag="hT")
                nc.vector.tensor_copy(out=hT[:], in_=pT[:])
                stage_a(li + 1, b, hT[:], htil_next)

            # ---- aggregate: PSUM chains interleaved across block pairs ----
            for b0 in range(0, NBLK, 2):
                pair = [b0] if b0 + 1 >= NBLK else [b0, b0 + 1]
                accs = {}
                mbs = {}
                for b in pair:
                    accs[b] = psA.tile(
                        [128, fo], mybir.dt.float32, space="PSUM", tag=f"acc{b%2}", name=f"acc_{li}_{b}"
                    )
                    mbs[b] = int(M_b[b])
                for j in range(max(mbs.values())):
                    for b in pair:
                        if j < mbs[b]:
                            S, mh = emit_chunk(b, j)
                            nc.tensor.matmul(
                                out=accs[b][:],
                                lhsT=S[:],
                                rhs=mh[:],
                                start=(j == 0),
                                stop=(j == mbs[b] - 1),
                            )
                for b in pair:
                    finish_block(b, accs[b])
            htil_cur = htil_next
        nc.sync.dma_start(out=out_ext[:], in_=out_sb[:])

    nc.finalize()
    return nc


LAST_EXEC_NS = None
LAST_RESULT = None


def kernel(x, edge_index, edge_weight, W1, b1, W2, b2, W3, b3, W4, b4, W5, b5, W6, b6, W7, b7):
    global LAST_EXEC_NS, LAST_RESULT
    import os

    from concourse.bass_utils import run_bass_kernel_spmd

    x = np.asarray(x, dtype=np.float32)
    in_maps, M_b, cum, D = _host_prep(x, np.asarray(edge_index), np.asarray(edge_weight))

    Wmats = [np.asarray(W, dtype=np.float32).astype(ml_dtypes.bfloat16) for W in (W1, W2, W3, W4, W5, W6, W7)]
    bvecs = [np.tile(np.asarray(b, dtype=np.float32).reshape(1, -1), (128, 1)) for b in (b1, b2, b3, b4, b5, b6, b7)]
    iota = np.tile(np.arange(128, dtype=np.float32).reshape(1, 128), (128, 1)).astype(ml_dtypes.bfloat16)
    ident = np.eye(128, dtype=np.float32).astype(ml_dtypes.bfloat16)
    for m in in_maps:
        for i in range(NLAYER):
            m[f"W{i+1}"] = Wmats[i]
            m[f"b{i+1}"] = bvecs[i]
        m["iota"] = iota
        m["ident"] = ident

    nc = _build_program(M_b, cum, D)
    trace = bool(int(os.environ.get("GCN_TRACE", "0")))
    res = run_bass_kernel_spmd(
        nc,
        in_maps,
        list(range(NC)),
        trace=trace,
        tmpdir=os.environ.get("GCN_TRACE_DIR") or None,
    )
    LAST_EXEC_NS = res.exec_time_ns
    LAST_RESULT = res
    outs = []
    for k in range(NC):
        pm = res.results[k]["out"]  # [128, NBLK], node c*128+p at [p, c]
        outs.append(pm.T.reshape(-1, 1)[:NLOC])
    out = np.concatenate(outs, axis=0)
    return out
